# revision 1
# baseline (speedup 1.0000x reference)
"""CoAttention (BiDAF-style) + depthwise-separable conv, Trainium2 Bass kernel.

Shapes (hardcoded): B=32, D=128, C_LEN=1024, Q_LEN=256.
Sharding: pure data-parallel over batch, 4 batches per core on 8 cores.

Math (masks enter only as additive -1e30 terms; row/col biases that are
constant along the softmax axis cancel, so S is never materialized with
both biases):
  S0[i,j]   = sum_k C_t[i,k] w3[k] Q_t[j,k]
  cb[j]     = Q_t[j].w2 (+ mask bias), rb[i] = C_t[i].w1 (+ mask bias)
  S_bar     = softmax_j(S0 + cb[j])      (rb cancels)
  S_bbar    = softmax_i(S0 + rb[i])      (cb cancels)
  A   = S_bar @ Q_t          (computed transposed: A^T, k-part x i-free)
  T   = S_bbar^T @ C_t       (j-part x k-free)
  Bm  = S_bar @ T            (computed transposed: Bm^T)
  x   = [C_t; A; C_t*A; C_t*Bm] channels (4*128, i)  -> depthwise conv5 -> pw conv
Softmax max-subtraction is skipped (|S| is O(5), exp is safe in fp32).
"""

import math
import os
from contextlib import ExitStack

import numpy as np

# The axon NTFF profile hook is not available in this container; a
# BASS_TRACE=1 leaking into the environment would crash the run path.
os.environ["BASS_NEVER_TRACE"] = "1"

import concourse.bass as bass
import concourse.mybir as mybir
import concourse.tile as tile
from concourse import bacc
from concourse.bass_utils import run_bass_kernel_spmd
from concourse.masks import make_identity

B, D, CL, QL = 32, 128, 1024, 256
NCORES = 8
BPC = B // NCORES  # batches per core
F32 = mybir.dt.float32
F32R = mybir.dt.float32r
BF16 = mybir.dt.bfloat16
AF = mybir.ActivationFunctionType
OP = mybir.AluOpType

NT_I = CL // 128  # 8 i-tiles
NT_J = QL // 128  # 2 j-tiles
NCH = CL // 512   # 2 n-chunks of 512

# knob: dtype used for matmul operands. float32r streams at 1 cycle/row
# (vs 4 for float32) but requires producer-side rounding; float32 is exact.
MM_DT = F32


def _mm(ap):
    """View an fp32 AP with the matmul operand dtype."""
    if MM_DT is F32R:
        return ap.bitcast(F32R)
    return ap


def build_kernel(wc_np: np.ndarray, pwT_np: np.ndarray):
    nc = bacc.Bacc("TRN2", target_bir_lowering=False, debug=False, num_devices=NCORES)

    C_in = nc.dram_tensor("C", [BPC, D, CL], F32, kind="ExternalInput")
    Q_in = nc.dram_tensor("Q", [BPC, D, QL], F32, kind="ExternalInput")
    cmb_in = nc.dram_tensor("cmb", [BPC, D, NT_I], F32, kind="ExternalInput")
    qmb_in = nc.dram_tensor("qmb", [BPC, D, NT_J], F32, kind="ExternalInput")
    out_d = nc.dram_tensor("out", [BPC, D, CL], F32, kind="ExternalOutput")

    wc_d = nc.inline_tensor(wc_np, "wc")      # (128, 25) packed consts
    pwT_d = nc.inline_tensor(pwT_np, "pwT")   # (512, 128) pw weights^T

    with tile.TileContext(nc) as tc, ExitStack() as ctx:
        consts = ctx.enter_context(tc.tile_pool(name="consts", bufs=1))
        sb = ctx.enter_context(tc.tile_pool(name="sb", bufs=2))
        psb = ctx.enter_context(tc.tile_pool(name="psb", bufs=3, space="PSUM"))
        pss = ctx.enter_context(tc.tile_pool(name="pss", bufs=2, space="PSUM"))

        wc = consts.tile([D, 25], F32)
        nc.sync.dma_start(out=wc, in_=wc_d[:, :])
        w1 = wc[:, 0:1]
        w2 = wc[:, 1:2]
        w3 = wc[:, 2:3]
        ones_col = wc[:, 3:4]
        fbias = wc[:, 4:5]

        pwT = consts.tile([D, 4, D], F32)
        nc.sync.dma_start(out=pwT, in_=pwT_d.rearrange("(g p) d -> p g d", p=D))
        ident = consts.tile([D, D], F32)
        make_identity(nc, ident)

        for b in range(BPC):
            # ---- loads (C goes into a border-padded tile: conv group 0) ----
            cbp = sb.tile([D, CL + 4], F32, tag="cbp")
            nc.gpsimd.memset(cbp[:, 0:2], 0.0)
            nc.gpsimd.memset(cbp[:, CL + 2 : CL + 4], 0.0)
            nc.sync.dma_start(out=cbp[:, 2 : CL + 2], in_=C_in[b])
            cb = cbp[:, 2 : CL + 2]
            qb = sb.tile([D, QL], F32, tag="qb")
            nc.sync.dma_start(out=qb, in_=Q_in[b])
            cmbt = sb.tile([D, NT_I], F32, tag="cmbt")
            nc.sync.dma_start(out=cmbt, in_=cmb_in[b])
            qmbt = sb.tile([D, NT_J], F32, tag="qmbt")
            nc.sync.dma_start(out=qmbt, in_=qmb_in[b])

            # ---- Qw3 = Q * w3 (per-partition scalar) ----
            qw3 = sb.tile([D, QL], F32, tag="qw3")
            nc.vector.tensor_scalar_mul(qw3, qb, w3)

            # ---- cb_col = Q_t @ w2 per j-tile, + mask bias ----
            ps_cb = pss.tile([D, NT_J], F32, tag="small")
            for jt in range(NT_J):
                nc.tensor.matmul(
                    ps_cb[:, jt : jt + 1],
                    _mm(qb[:, jt * 128 : (jt + 1) * 128]),
                    _mm(w2),
                    start=True,
                    stop=True,
                )
            cbm = sb.tile([D, NT_J], F32, tag="cbm")
            nc.vector.tensor_add(cbm, ps_cb, qmbt)

            # ---- S0^T (j-part, i-free) and E^T = exp(S0^T + cb[j]) ----
            et = []
            for jt in range(NT_J):
                s0t = psb.tile([D, CL], F32, tag="big")
                for n in range(NCH):
                    nc.tensor.matmul(
                        s0t[:, n * 512 : (n + 1) * 512],
                        _mm(qw3[:, jt * 128 : (jt + 1) * 128]),
                        _mm(cb[:, n * 512 : (n + 1) * 512]),
                        start=True,
                        stop=True,
                    )
                e = sb.tile([D, CL], F32, tag="et")
                nc.scalar.activation(e, s0t, AF.Exp, bias=cbm[:, jt : jt + 1])
                et.append(e)

            # ---- S0 i-part (for S_bbar) + row bias columns ----
            ps_rb = pss.tile([D, NT_I], F32, tag="small")
            s0ip = []
            for h in range(2):
                s0ip.append(psb.tile([D, CL], F32, tag="big", name=f"s0ip{h}"))
            for m in range(NT_I):
                h, m4 = divmod(m, 4)
                nc.tensor.matmul(
                    s0ip[h][:, m4 * 256 : (m4 + 1) * 256],
                    _mm(cb[:, m * 128 : (m + 1) * 128]),
                    _mm(qw3),
                    start=True,
                    stop=True,
                )
                nc.tensor.matmul(
                    ps_rb[:, m : m + 1],
                    _mm(cb[:, m * 128 : (m + 1) * 128]),
                    _mm(w1),
                    start=True,
                    stop=True,
                )
            rbm = sb.tile([D, NT_I], F32, tag="rbm")
            nc.vector.tensor_add(rbm, ps_rb, cmbt)
            exprb = sb.tile([D, NT_I], F32, tag="exprb")
            nc.scalar.activation(exprb, rbm, AF.Exp)
            sbb = []
            for h in range(2):
                s = sb.tile([D, CL], F32, tag="sbb")
                nc.scalar.activation(s, s0ip[h], AF.Exp)
                sbb.append(s)

            # ---- rs = sum_j E^T  (ones-matmul), r = 1/rs, broadcast ----
            rs = [
                pss.tile([1, 512], F32, tag="small", name=f"rs{n}") for n in range(NCH)
            ]
            for n in range(NCH):
                for jt in range(NT_J):
                    nc.tensor.matmul(
                        rs[n][0:1, :],
                        _mm(ones_col),
                        _mm(et[jt][:, n * 512 : (n + 1) * 512]),
                        start=(jt == 0),
                        stop=(jt == NT_J - 1),
                    )
            rrow = sb.tile([1, CL], F32, tag="rrow")
            for n in range(NCH):
                nc.vector.reciprocal(rrow[0:1, n * 512 : (n + 1) * 512], rs[n][0:1, :])
            Rb = sb.tile([D, CL], F32, tag="Rb")
            nc.gpsimd.partition_broadcast(Rb, rrow)

            # ---- C^T tiles via PE transpose, scaled by exp(rb) ----
            # cbt[:, ich, 0:128] = exp(rb_i) * C_t[i, :]; col 128 = exp(rb_i)
            cbt = sb.tile([D, NT_I, 129], F32, tag="cbt")
            for ich in range(NT_I):
                pt = pss.tile([D, D], F32, tag="small")
                nc.tensor.transpose(pt, cb[:, ich * 128 : (ich + 1) * 128], ident)
                nc.scalar.mul(cbt[:, ich, 0:128], pt, exprb[:, ich : ich + 1])
                nc.gpsimd.tensor_copy(cbt[:, ich, 128:129], exprb[:, ich : ich + 1])

            # ---- T = S_bbar^T @ C_t with built-in denominator column ----
            tT = []
            for jt in range(NT_J):
                ps_t = pss.tile([D, 129], F32, tag="small")
                for ich in range(NT_I):
                    h, m4 = divmod(ich, 4)
                    nc.tensor.matmul(
                        ps_t,
                        _mm(sbb[h][:, m4 * 256 + jt * 128 : m4 * 256 + (jt + 1) * 128]),
                        _mm(cbt[:, ich, :]),
                        start=(ich == 0),
                        stop=(ich == NT_I - 1),
                    )
                rt = sb.tile([D, 1], F32, tag="rt")
                nc.vector.reciprocal(rt, ps_t[:, 128:129])
                t_sb = sb.tile([D, D], F32, tag="tsb")
                nc.scalar.mul(t_sb, ps_t[:, 0:128], rt)
                tT.append(t_sb)

            # ---- Q_t tiles via PE transpose ----
            qbt = sb.tile([D, NT_J, D], F32, tag="qbt")
            for jt in range(NT_J):
                pt2 = pss.tile([D, D], F32, tag="small")
                nc.tensor.transpose(pt2, qb[:, jt * 128 : (jt + 1) * 128], ident)
                nc.scalar.copy(qbt[:, jt, :], pt2)

            # ---- A^T and Bm^T (contract over j) ----
            ps_a = psb.tile([D, CL], F32, tag="big")
            for n in range(NCH):
                for jt in range(NT_J):
                    nc.tensor.matmul(
                        ps_a[:, n * 512 : (n + 1) * 512],
                        _mm(qbt[:, jt, :]),
                        _mm(et[jt][:, n * 512 : (n + 1) * 512]),
                        start=(jt == 0),
                        stop=(jt == NT_J - 1),
                    )
            ps_b = psb.tile([D, CL], F32, tag="big")
            for n in range(NCH):
                for jt in range(NT_J):
                    nc.tensor.matmul(
                        ps_b[:, n * 512 : (n + 1) * 512],
                        _mm(tT[jt]),
                        _mm(et[jt][:, n * 512 : (n + 1) * 512]),
                        start=(jt == 0),
                        stop=(jt == NT_J - 1),
                    )

            # ---- conv input channel groups (padded for the 5-tap conv) ----
            g1 = sb.tile([D, CL + 4], F32, tag="g1")
            g2 = sb.tile([D, CL + 4], F32, tag="g2")
            g3 = sb.tile([D, CL + 4], F32, tag="g3")
            for g in (g1, g2, g3):
                nc.gpsimd.memset(g[:, 0:2], 0.0)
                nc.gpsimd.memset(g[:, CL + 2 : CL + 4], 0.0)
            tmp3 = sb.tile([D, CL], F32, tag="tmp3")
            nc.vector.tensor_mul(g1[:, 2 : CL + 2], ps_a, Rb)
            nc.vector.tensor_mul(g2[:, 2 : CL + 2], g1[:, 2 : CL + 2], cb)
            nc.vector.tensor_mul(tmp3, ps_b, Rb)
            nc.vector.tensor_mul(g3[:, 2 : CL + 2], tmp3, cb)

            # ---- depthwise conv5 + pointwise conv (fused bias at the end) ----
            ps_o = psb.tile([D, CL], F32, tag="big")
            for g, xg in enumerate((cbp, g1, g2, g3)):
                dwo = sb.tile([D, CL], F32, tag="dwo")
                wcol = lambda t: wc[:, 5 + g * 5 + t : 6 + g * 5 + t]
                nc.vector.tensor_scalar_mul(dwo, xg[:, 0:CL], wcol(0))
                for t in range(1, 5):
                    eng = nc.vector
                    eng.scalar_tensor_tensor(
                        dwo, xg[:, t : t + CL], wcol(t), dwo, OP.mult, OP.add
                    )
                for n in range(NCH):
                    nc.tensor.matmul(
                        ps_o[:, n * 512 : (n + 1) * 512],
                        _mm(pwT[:, g, :]),
                        _mm(dwo[:, n * 512 : (n + 1) * 512]),
                        start=(g == 0),
                        stop=(g == 3),
                    )

            outsb = sb.tile([D, CL], F32, tag="outsb")
            nc.scalar.activation(outsb, ps_o, AF.Identity, bias=fbias)
            nc.sync.dma_start(out=out_d[b], in_=outsb)

    nc.compile()
    return nc


def _host_prep(c_mask, q_mask, W0, dw_w, dw_b, pw_w, pw_b):
    w1, w2, w3 = W0[:D], W0[D : 2 * D], W0[2 * D :]
    wc = np.zeros((D, 25), np.float32)
    wc[:, 0] = w1
    wc[:, 1] = w2
    wc[:, 2] = w3
    wc[:, 3] = 1.0
    pw = pw_w[:, :, 0].astype(np.float32)  # (128, 512)
    wc[:, 4] = pw @ dw_b + pw_b
    dw = dw_w[:, 0, :].reshape(4, D, 5).astype(np.float32)
    wc[:, 5:25] = dw.transpose(1, 0, 2).reshape(D, 20)
    pwT = np.ascontiguousarray(pw.T)  # (512, 128)
    cmb = np.ascontiguousarray(
        ((c_mask - 1.0) * 1e30).reshape(B, NT_I, D).transpose(0, 2, 1)
    ).astype(np.float32)
    qmb = np.ascontiguousarray(
        ((q_mask - 1.0) * 1e30).reshape(B, NT_J, D).transpose(0, 2, 1)
    ).astype(np.float32)
    return wc, pwT, cmb, qmb


def kernel(C, Q, c_mask, q_mask, W0, dw_w, dw_b, pw_w, pw_b):
    C = np.ascontiguousarray(np.asarray(C, np.float32))
    Q = np.ascontiguousarray(np.asarray(Q, np.float32))
    wc, pwT, cmb, qmb = _host_prep(
        np.asarray(c_mask, np.float32),
        np.asarray(q_mask, np.float32),
        np.asarray(W0, np.float32),
        np.asarray(dw_w, np.float32),
        np.asarray(dw_b, np.float32),
        np.asarray(pw_w, np.float32),
        np.asarray(pw_b, np.float32),
    )
    nc = build_kernel(wc, pwT)
    in_maps = []
    for c in range(NCORES):
        sl = slice(c * BPC, (c + 1) * BPC)
        in_maps.append(
            {
                "C": np.ascontiguousarray(C[sl]),
                "Q": np.ascontiguousarray(Q[sl]),
                "cmb": np.ascontiguousarray(cmb[sl]),
                "qmb": np.ascontiguousarray(qmb[sl]),
            }
        )
    res = run_bass_kernel_spmd(nc, in_maps, core_ids=list(range(NCORES)))
    global LAST_RESULT, LAST_NC, LAST_IN_MAPS
    LAST_RESULT, LAST_NC, LAST_IN_MAPS = res, nc, in_maps
    out = np.concatenate([r["out"] for r in res.results], axis=0)
    return out.astype(np.float32)


LAST_RESULT = None
LAST_NC = None
LAST_IN_MAPS = None



# revision 5
# speedup vs baseline: 1.9129x; 1.9129x over previous
"""CoAttention (BiDAF-style) + depthwise-separable conv, Trainium2 Bass kernel.

Shapes (hardcoded): B=32, D=128, C_LEN=1024, Q_LEN=256.
Sharding: pure data-parallel over batch, 4 batches per core on 8 cores.

v2: bf16 matmul operands (1 cyc/row vs 4 for fp32), exp(rb) folded into the
column-softmax exp via per-partition bias, softmax normalization folded into
E^T before the A/Bm matmuls, and the depthwise conv split between the PE
(im2col-style fused taps, weights pre-multiplied with the pointwise conv) and
the DVE (per-tap tensor_scalar muls at 4x + adds at 2x), balancing engine
occupancy.

Math (masks enter only as additive -1e30 terms; biases constant along a
softmax axis cancel):
  S0[i,j]  = sum_k C_t[i,k] w3[k] Q_t[j,k]
  cb[j]    = Q_t[j].w2 (+ mask), rb[i] = C_t[i].w1 (+ mask)
  E^T      = exp(S0^T + cb[j])               (j-part, i-free)
  r[i]     = 1 / sum_j E^T;  En = E^T * r    -> S_bar rows, normalized
  sbb      = exp(S0 + rb[i])                 (i-part, j-free), includes rb
  T[j,k]   = (sum_i sbb * C^T[i,k]) / (sum_i sbb)   (denominator via ones col)
  A_n^T    = Q^T_tiles @ En      Bm_n^T = T_tiles @ En
  x groups = [C; A_n; C*A_n; C*Bm_n] -> conv5 depthwise -> pointwise + bias
"""

import math
import os
from contextlib import ExitStack

import numpy as np

os.environ["BASS_NEVER_TRACE"] = "1"

import concourse.bass as bass
import concourse.mybir as mybir
import concourse.tile as tile
from concourse import bacc
from concourse.bass_utils import run_bass_kernel_spmd
from concourse.masks import make_identity

B, D, CL, QL = 32, 128, 1024, 256
NCORES = 8
BPC = B // NCORES
F32 = mybir.dt.float32
BF16 = mybir.dt.bfloat16
AF = mybir.ActivationFunctionType
OP = mybir.AluOpType

NT_I = CL // 128  # 8 i-tiles
NT_J = QL // 128  # 2 j-tiles
NCH = CL // 512   # 2 col-chunks of 512

# number of conv taps computed on DVE (rest fused into PE matmuls), per group
DVE_TAPS = (2, 2, 2, 3)


def build_kernel(wc_np, wcb_np, wconv_np, pwt_np):
    nc = bacc.Bacc("TRN2", target_bir_lowering=False, debug=False, num_devices=NCORES)

    C_in = nc.dram_tensor("C", [BPC, D, CL], F32, kind="ExternalInput")
    Q_in = nc.dram_tensor("Q", [BPC, D, QL], F32, kind="ExternalInput")
    mb_in = nc.dram_tensor("mb", [BPC, D, NT_I + NT_J], F32, kind="ExternalInput")
    out_d = nc.dram_tensor("out", [BPC, D, CL], F32, kind="ExternalOutput")

    wc_d = nc.inline_tensor(wc_np, "wc")        # (128, 25) fp32 scalars
    wcb_d = nc.inline_tensor(wcb_np, "wcb")     # (128, 3) bf16 matmul consts
    wconv_d = nc.inline_tensor(wconv_np, "wconv")  # (128, 20, 128) bf16 PE conv
    pwt_d = nc.inline_tensor(pwt_np, "pwt")     # (128, 4, 128) bf16 pw^T

    with tile.TileContext(nc) as tc, ExitStack() as ctx:
        consts = ctx.enter_context(tc.tile_pool(name="consts", bufs=1))
        sb = ctx.enter_context(tc.tile_pool(name="sb", bufs=2))
        psf = ctx.enter_context(tc.tile_pool(name="psf", bufs=3, space="PSUM"))
        psa = ctx.enter_context(tc.tile_pool(name="psa", bufs=2, space="PSUM"))

        wc = consts.tile([D, 25], F32)
        nc.sync.dma_start(out=wc, in_=wc_d[:, :])
        w3 = wc[:, 2:3]
        fbias = wc[:, 4:5]

        wcb = consts.tile([D, 3], BF16)
        nc.sync.dma_start(out=wcb, in_=wcb_d[:, :])
        w1b = wcb[:, 0:1]
        w2b = wcb[:, 1:2]
        onesb = wcb[:, 2:3]

        wconv = consts.tile([D, 20, D], BF16)
        nc.sync.dma_start(out=wconv, in_=wconv_d[:, :, :])
        pwt = consts.tile([D, 4, D], BF16)
        nc.sync.dma_start(out=pwt, in_=pwt_d[:, :, :])

        ident = consts.tile([D, D], BF16)
        make_identity(nc, ident)

        for b in range(BPC):
            first = b < 2  # pool bufs=2: per-buffer one-time initialization

            # ---- loads ----
            cfp = sb.tile([D, CL], F32, tag="cfp")
            nc.sync.dma_start(out=cfp, in_=C_in[b])
            qb = sb.tile([D, QL], BF16, tag="qb")
            nc.gpsimd.dma_start(out=qb, in_=Q_in[b])  # casting DMA (SWDGE)
            mbt = sb.tile([D, NT_I + NT_J], F32, tag="mbt")
            nc.sync.dma_start(out=mbt, in_=mb_in[b])

            # C in bf16, padded for the conv (border zeros set once per buf)
            cbp = sb.tile([D, CL + 4], BF16, tag="cbp")
            if first:
                nc.gpsimd.memset(cbp[:, 0:2], 0.0)
                nc.gpsimd.memset(cbp[:, CL + 2 : CL + 4], 0.0)
            nc.vector.tensor_copy(cbp[:, 2 : CL + 2], cfp)  # fp32->bf16, 2x
            cb = cbp[:, 2 : CL + 2]

            # ---- qw3 = Q * w3 ----
            qw3 = sb.tile([D, QL], BF16, tag="qw3")
            nc.vector.tensor_scalar_mul(qw3, qb, w3)

            # ---- cb_col = Q_t @ w2 + q-mask bias ----
            ps_cb = psa.tile([D, NT_J], F32, tag="aux", name="pscb")
            for jt in range(NT_J):
                nc.tensor.matmul(
                    ps_cb[:, jt : jt + 1],
                    qb[:, jt * 128 : (jt + 1) * 128],
                    w2b,
                    start=True,
                    stop=True,
                )
            cbm = sb.tile([D, NT_J], F32, tag="cbm")
            nc.vector.tensor_add(cbm, ps_cb, mbt[:, NT_I : NT_I + NT_J])

            # ---- rb row bias ----
            ps_rb = psa.tile([D, NT_I], F32, tag="aux", name="psrb")
            for m in range(NT_I):
                nc.tensor.matmul(
                    ps_rb[:, m : m + 1],
                    cb[:, m * 128 : (m + 1) * 128],
                    w1b,
                    start=True,
                    stop=True,
                )
            rbm = sb.tile([D, NT_I], F32, tag="rbm")
            nc.vector.tensor_add(rbm, ps_rb, mbt[:, 0:NT_I])

            # ---- E^T = exp(S0^T + cb[j]) ----
            et = []
            for jt in range(NT_J):
                s0t = psf.tile([D, CL], F32, tag="flow", name=f"s0t{jt}")
                for n in range(NCH):
                    nc.tensor.matmul(
                        s0t[:, n * 512 : (n + 1) * 512],
                        qw3[:, jt * 128 : (jt + 1) * 128],
                        cb[:, n * 512 : (n + 1) * 512],
                        start=True,
                        stop=True,
                    )
                e = sb.tile([D, CL], BF16, tag="et")
                nc.scalar.activation(e, s0t, AF.Exp, bias=cbm[:, jt : jt + 1])
                et.append(e)

            # ---- sbb = exp(S0 + rb[i])  (i-part, j-free; rb via bias) ----
            sbb = []
            for h in range(2):
                ip = psf.tile([D, CL], F32, tag="flow", name=f"ip{h}")
                for m4 in range(4):
                    m = h * 4 + m4
                    nc.tensor.matmul(
                        ip[:, m4 * 256 : (m4 + 1) * 256],
                        cb[:, m * 128 : (m + 1) * 128],
                        qw3,
                        start=True,
                        stop=True,
                    )
                s = sb.tile([D, 4, QL], BF16, tag="sbb")
                for m4 in range(4):
                    m = h * 4 + m4
                    nc.scalar.activation(
                        s[:, m4, :],
                        ip[:, m4 * 256 : (m4 + 1) * 256],
                        AF.Exp,
                        bias=rbm[:, m : m + 1],
                    )
                sbb.append(s)

            # ---- r = 1/colsum(E^T); En = E^T * r (normalized in place) ----
            rrow = sb.tile([1, CL], BF16, tag="rrow")
            for n in range(NCH):
                rs = psa.tile([1, 512], F32, tag="aux", name=f"rs{n}")
                for jt in range(NT_J):
                    nc.tensor.matmul(
                        rs,
                        onesb,
                        et[jt][:, n * 512 : (n + 1) * 512],
                        start=(jt == 0),
                        stop=(jt == NT_J - 1),
                    )
                with nc.allow_low_precision(
                    reason="softmax denom reciprocal in bf16; uniform per-"
                    "column scale, well within the 2e-2 tolerance"
                ):
                    nc.vector.reciprocal(rrow[0:1, n * 512 : (n + 1) * 512], rs)
            Rb = sb.tile([D, CL], BF16, tag="Rb")
            nc.gpsimd.partition_broadcast(Rb, rrow)
            for jt in range(NT_J):
                nc.vector.tensor_mul(et[jt], et[jt], Rb)

            # ---- C^T tiles (PE transpose, bf16) + ones denominator col ----
            cbt = []
            for h in range(2):
                ct = psa.tile([D, 4, D], BF16, tag="aux", name=f"ct{h}")
                for m4 in range(4):
                    m = h * 4 + m4
                    nc.tensor.transpose(
                        ct[:, m4, :], cb[:, m * 128 : (m + 1) * 128], ident
                    )
                c = sb.tile([D, 4, D + 1], BF16, tag="cbt")
                if first:
                    nc.gpsimd.memset(c[:, :, D : D + 1], 1.0)
                nc.vector.tensor_copy(c[:, :, 0:D], ct)
                cbt.append(c)

            # ---- Q^T tiles ----
            qt = psa.tile([D, NT_J, D], BF16, tag="aux", name="qt")
            for jt in range(NT_J):
                nc.tensor.transpose(qt[:, jt, :], qb[:, jt * 128 : (jt + 1) * 128], ident)
            qbt = sb.tile([D, NT_J, D], BF16, tag="qbt")
            nc.vector.tensor_copy(qbt, qt)

            # ---- T = (S_bbar^T @ C_t) with built-in denominator ----
            tT = []
            for jt in range(NT_J):
                ps_t = psa.tile([D, D + 1], F32, tag="aux", name=f"t{jt}")
                for ich in range(NT_I):
                    h, m4 = divmod(ich, 4)
                    nc.tensor.matmul(
                        ps_t,
                        sbb[h][:, m4, jt * 128 : (jt + 1) * 128],
                        cbt[h][:, m4, :],
                        start=(ich == 0),
                        stop=(ich == NT_I - 1),
                    )
                rt = sb.tile([D, 1], F32, tag="rt")
                nc.vector.reciprocal(rt, ps_t[:, D : D + 1])
                tsb = sb.tile([D, D], BF16, tag="tsb")
                nc.vector.tensor_scalar_mul(tsb, ps_t[:, 0:D], rt)
                tT.append(tsb)

            # ---- A_n^T and Bm_n^T (contract over j; En already normalized) --
            ps_a = psf.tile([D, CL], F32, tag="flow", name="psa")
            for n in range(NCH):
                for jt in range(NT_J):
                    nc.tensor.matmul(
                        ps_a[:, n * 512 : (n + 1) * 512],
                        qbt[:, jt, :],
                        et[jt][:, n * 512 : (n + 1) * 512],
                        start=(jt == 0),
                        stop=(jt == NT_J - 1),
                    )
            ps_b = psf.tile([D, CL], F32, tag="flow", name="psb")
            for n in range(NCH):
                for jt in range(NT_J):
                    nc.tensor.matmul(
                        ps_b[:, n * 512 : (n + 1) * 512],
                        tT[jt],
                        et[jt][:, n * 512 : (n + 1) * 512],
                        start=(jt == 0),
                        stop=(jt == NT_J - 1),
                    )

            # ---- conv input groups (padded bf16 tiles) ----
            g1 = sb.tile([D, CL + 4], BF16, tag="g1")
            g2 = sb.tile([D, CL + 4], BF16, tag="g2")
            g3 = sb.tile([D, CL + 4], BF16, tag="g3")
            if first:
                for g in (g1, g2, g3):
                    nc.gpsimd.memset(g[:, 0:2], 0.0)
                    nc.gpsimd.memset(g[:, CL + 2 : CL + 4], 0.0)
            nc.scalar.copy(g1[:, 2 : CL + 2], ps_a)              # A_n (ACT)
            nc.scalar.copy(g3[:, 2 : CL + 2], ps_b)              # Bm_n (ACT)
            nc.vector.tensor_mul(g2[:, 2 : CL + 2], g1[:, 2 : CL + 2], cb)
            nc.vector.tensor_mul(g3[:, 2 : CL + 2], g3[:, 2 : CL + 2], cb)

            groups = (cbp, g1, g2, g3)

            # ---- DVE partial depthwise (first DVE_TAPS[g] taps per group) --
            dwo = []
            for g in range(4):
                nt = DVE_TAPS[g]
                if nt == 0:
                    dwo.append(None)
                    continue
                xg = groups[g]
                m = sb.tile([D, CL], BF16, tag=f"dw{g}")
                nc.vector.tensor_scalar_mul(m, xg[:, 0:CL], wc[:, 5 + g * 5 : 6 + g * 5])
                for t in range(1, nt):
                    mt = sb.tile([D, CL], BF16, tag=f"dwt{g}")
                    nc.vector.tensor_scalar_mul(
                        mt, xg[:, t : t + CL], wc[:, 5 + g * 5 + t : 6 + g * 5 + t]
                    )
                    nc.vector.tensor_add(m, m, mt)
                dwo.append(m)

            # ---- fused pointwise: PSUM-accumulate pw@dwo + PE conv taps ----
            ps_o = psf.tile([D, CL], F32, tag="flow", name="pso")
            for n in range(NCH):
                started = False
                for g in range(4):
                    nt = DVE_TAPS[g]
                    if nt > 0:
                        nc.tensor.matmul(
                            ps_o[:, n * 512 : (n + 1) * 512],
                            pwt[:, g, :],
                            dwo[g][:, n * 512 : (n + 1) * 512],
                            start=not started,
                            stop=False,
                        )
                        started = True
                    for t in range(nt, 5):
                        nc.tensor.matmul(
                            ps_o[:, n * 512 : (n + 1) * 512],
                            wconv[:, g * 5 + t, :],
                            groups[g][:, t + n * 512 : t + n * 512 + 512],
                            start=not started,
                            stop=(g == 3 and t == 4),
                        )
                        started = True

            outsb = sb.tile([D, CL], F32, tag="outsb")
            nc.scalar.activation(outsb, ps_o, AF.Identity, bias=fbias)
            nc.sync.dma_start(out=out_d[b], in_=outsb)

    nc.compile()
    return nc


def _host_prep(c_mask, q_mask, W0, dw_w, dw_b, pw_w, pw_b):
    w1, w2, w3 = W0[:D], W0[D : 2 * D], W0[2 * D :]
    wc = np.zeros((D, 25), np.float32)
    wc[:, 0] = w1
    wc[:, 1] = w2
    wc[:, 2] = w3
    wc[:, 3] = 1.0
    pw = pw_w[:, :, 0].astype(np.float32)  # (128, 512)
    wc[:, 4] = pw @ dw_b + pw_b
    dw = dw_w[:, 0, :].reshape(4, D, 5).astype(np.float32)  # (g, c, t)
    wc[:, 5:25] = dw.transpose(1, 0, 2).reshape(D, 20)  # per-channel scalars

    wcb = np.zeros((D, 3), np.float32)
    wcb[:, 0] = w1
    wcb[:, 1] = w2
    wcb[:, 2] = 1.0

    # PE conv stationaries: wconv[(g*5+t)][d, c] = pw[d, g*128+c] * dw[g,c,t]
    # matmul computes lhsT.T @ rhs with lhsT (contraction=c, out=d) so store
    # transposed: wconv_np[c, g*5+t, d] = pw[d, g*128+c] * dw[g, c, t]
    wconv = np.zeros((D, 20, D), np.float32)
    for g in range(4):
        for t in range(5):
            wconv[:, g * 5 + t, :] = (pw[:, g * D : (g + 1) * D] * dw[g, :, t]).T
    pwt = np.zeros((D, 4, D), np.float32)
    for g in range(4):
        pwt[:, g, :] = pw[:, g * D : (g + 1) * D].T

    mb = np.concatenate(
        [
            ((c_mask - 1.0) * 1e30).reshape(B, NT_I, D).transpose(0, 2, 1),
            ((q_mask - 1.0) * 1e30).reshape(B, NT_J, D).transpose(0, 2, 1),
        ],
        axis=2,
    ).astype(np.float32)

    import ml_dtypes

    tobf = lambda a: a.astype(ml_dtypes.bfloat16)
    return wc, tobf(wcb), tobf(wconv), tobf(pwt), np.ascontiguousarray(mb)


def kernel(C, Q, c_mask, q_mask, W0, dw_w, dw_b, pw_w, pw_b):
    C = np.ascontiguousarray(np.asarray(C, np.float32))
    Q = np.ascontiguousarray(np.asarray(Q, np.float32))
    wc, wcb, wconv, pwt, mb = _host_prep(
        np.asarray(c_mask, np.float32),
        np.asarray(q_mask, np.float32),
        np.asarray(W0, np.float32),
        np.asarray(dw_w, np.float32),
        np.asarray(dw_b, np.float32),
        np.asarray(pw_w, np.float32),
        np.asarray(pw_b, np.float32),
    )
    nc = build_kernel(wc, wcb, wconv, pwt)
    in_maps = []
    for c in range(NCORES):
        sl = slice(c * BPC, (c + 1) * BPC)
        in_maps.append(
            {
                "C": np.ascontiguousarray(C[sl]),
                "Q": np.ascontiguousarray(Q[sl]),
                "mb": np.ascontiguousarray(mb[sl]),
            }
        )
    res = run_bass_kernel_spmd(nc, in_maps, core_ids=list(range(NCORES)))
    global LAST_RESULT, LAST_NC, LAST_IN_MAPS
    LAST_RESULT, LAST_NC, LAST_IN_MAPS = res, nc, in_maps
    out = np.concatenate([r["out"] for r in res.results], axis=0)
    return out.astype(np.float32)


LAST_RESULT = None
LAST_NC = None
LAST_IN_MAPS = None
